# revision 26
# baseline (speedup 1.0000x reference)
"""AGNN layer (cosine-attention message passing) on 8 TRN2 NeuronCores.

Host sharding: append self-loops, sort edges by destination node, cut the node
range into blocks (<=128 nodes, bounded edge count), hand contiguous block
runs to the 8 cores. Every softmax segment then lives on one core: no
collectives anywhere.

Device kernel per core (identical SPMD graph, per-core data):
  prologue: cast x (f32) -> bf16 src gather table; normalize the core's own
            node-range slice (xdst input) -> unit-norm bf16 dst table.
  per block: dma_gather (4 SWDGE queues round-robin, <=1024 rows/call)
            fetches per-edge rows: src rows from the low half [0,32768) and
            high half [32768,N) of the table (int16 index limit forces the
            static lo/hi tile split), dst rows from the per-core dst table
            (relative indices). Block-wide DVE/ACT ops compute per-edge
            cos = (x_s . xn_d) * rsqrt(|x_s|^2) and w = exp(beta*cos), and
            build the one-hot(dst)*w matrix via iota/is_equal; per 128-edge
            tile, two PSUM-accumulating matmuls build per-node sum(w*x_src)
            and sum(w).
  per block epilogue: rows = relu(M/s) written to a compact per-block
            output; the host scatters block rows back to node order.

Logits are cosines scaled by beta (bounded), so exp never overflows and the
reference's segment-max subtraction cancels exactly -- single pass suffices.
"""

import numpy as np
import ml_dtypes

import concourse.bacc as bacc
import concourse.mybir as mybir
import concourse.tile as tile
from concourse.library_config import mlp as _mlp_lib
import concourse.tile_sem_assignment as _tsa

# Tile's DMASW-lane round-robin is SWDGE-queue-oblivious; a completion sem
# shared by two queues desyncs the ucode's per-queue ring-reclaim waits.
# Partition the 8 lanes so queue q owns lanes {2q, 2q+1}.
_orig_assign_tick = _tsa.TileClockTick._assign_tick


def _assign_tick_queue_aware(self, inst):
    if (
        isinstance(inst, mybir.InstDMAGatherAnt)
        and inst.engine == mybir.EngineType.Pool
    ):
        qn = inst.queue_num
        if not hasattr(self, "_qcnt"):
            self._qcnt = {}
        cnt = self._qcnt.get(qn, 0)
        self._qcnt[qn] = cnt + 1
        self.next_sw_dma_idx = qn * 2 + (cnt % 2)
    return _orig_assign_tick(self, inst)


_tsa.TileClockTick._assign_tick = _assign_tick_queue_aware

P = 128
N_NODES = 50000
D_FEAT = 128
NCORES = 8
HI_BASE = 32768  # int16 gather index limit
TPB_LO = 23  # tiles per block for src<HI_BASE edges
TPB_HI = 12  # tiles per block for src>=HI_BASE edges
BLK_SENTINEL = 300.0

F32 = mybir.dt.float32
BF16 = mybir.dt.bfloat16
I16 = mybir.dt.int16


def _build_graph(N, D, NB, tpb_lo, tpb_hi, ndst, hi_base):
    tpb = tpb_lo + tpb_hi
    gl, gh, gt = tpb_lo * 8, tpb_hi * 8, tpb * 8  # idx cols (16-wrapped)
    nc = bacc.Bacc(
        "TRN2", target_bir_lowering=False, debug=False, enable_asserts=False,
        num_swdge_queues=4,
    )
    x_ext = nc.dram_tensor("x", [N, D], F32, kind="ExternalInput").ap()
    xdst_ext = nc.dram_tensor("xdst", [ndst, D], F32, kind="ExternalInput").ap()
    ga = gl + gh + gt
    idx_all = nc.dram_tensor("idx_all", [P, NB * ga], I16, kind="ExternalInput").ap()
    dst_blk = nc.dram_tensor("dst_blk", [P, NB * tpb], BF16, kind="ExternalInput").ap()
    beta128 = nc.dram_tensor("beta128", [P, 1], F32, kind="ExternalInput").ap()
    out_ext = nc.dram_tensor("out", [NB * P, D], F32, kind="ExternalOutput").ap()

    n_lo = min(hi_base, N)
    n_hi = max(N - hi_base, 8)
    xb_lo = nc.dram_tensor("xb_lo", [n_lo, D], BF16).ap()
    xb_hi = nc.dram_tensor("xb_hi", [n_hi, D], BF16).ap()
    xd_tab = nc.dram_tensor("xd_table", [ndst, D], BF16).ap()

    with tile.TileContext(nc) as tc:
        with (
            tc.tile_pool(name="const", bufs=1) as constp,
            tc.tile_pool(name="prolog", bufs=3) as prologp,
            tc.tile_pool(name="idx", bufs=3) as idxp,
            tc.tile_pool(name="gsl", bufs=3) as gslp,
            tc.tile_pool(name="gsh", bufs=3) as gshp,
            tc.tile_pool(name="gd", bufs=3) as gdp,
            tc.tile_pool(name="work", bufs=2) as workp,
            tc.tile_pool(name="pw", bufs=3) as pwp,
            tc.tile_pool(name="cols", bufs=4) as colp,
            tc.tile_pool(name="orow", bufs=2) as orowp,
            tc.tile_pool(name="psum", bufs=4, space="PSUM") as psump,
        ):
            nc.gpsimd.load_library(_mlp_lib)

            # ---- constants ----
            iota_i16 = constp.tile([P, P], I16)
            nc.gpsimd.iota(iota_i16[:], pattern=[[1, P]], base=0, channel_multiplier=0)
            iota_bf = constp.tile([P, P], BF16)
            nc.vector.tensor_copy(iota_bf[:], iota_i16[:])
            ones_bf = constp.tile([P, 1], BF16)
            nc.vector.memset(ones_bf[:], 1.0)
            beta_sb = constp.tile([P, 1], F32)
            nc.sync.dma_start(out=beta_sb[:], in_=beta128[:, :])

            # ---- prologue: cast x -> bf16 tables ----
            def cast_table(src_ap, dst_ap, nrows):
                r0 = 0
                while r0 < nrows:
                    rows = min(2048, nrows - r0)
                    rpp = 16
                    while rows % rpp:
                        rpp //= 2
                    pp = rows // rpp
                    xt = prologp.tile([P, 16, D], F32, tag="xt")
                    nc.sync.dma_start(
                        out=xt[:pp, 0:rpp, :], in_=src_ap[r0 : r0 + rows, :]
                    )
                    xbt = prologp.tile([P, 16, D], BF16, tag="xbt")
                    nc.vector.tensor_copy(xbt[:pp, 0:rpp, :], xt[:pp, 0:rpp, :])
                    nc.scalar.dma_start(
                        out=dst_ap[r0 : r0 + rows, :], in_=xbt[:pp, 0:rpp, :]
                    )
                    r0 += rows

            # dst table is pre-normalized: xd_tab rows = x / |x|
            r0 = 0
            while r0 < ndst:
                rows = min(1024, ndst - r0)
                pp = rows // 8
                xt = prologp.tile([P, 8, D], F32, tag="xt")
                nc.sync.dma_start(out=xt[:pp], in_=xdst_ext[r0 : r0 + rows, :])
                sqv = prologp.tile([P, 8, D], F32, tag="sqv")
                nc.vector.tensor_tensor(
                    out=sqv[:pp], in0=xt[:pp], in1=xt[:pp], op=mybir.AluOpType.mult
                )
                ssum = prologp.tile([P, 8], F32, tag="ssum")
                nc.vector.tensor_reduce(
                    out=ssum[:pp], in_=sqv[:pp], axis=mybir.AxisListType.X,
                    op=mybir.AluOpType.add,
                )
                # rows of zeros (tail padding) -> clamp to avoid inf
                nsafe = prologp.tile([P, 8], F32, tag="nsafe")
                nc.vector.tensor_scalar(
                    out=nsafe[:pp], in0=ssum[:pp], scalar1=1e-30, scalar2=None,
                    op0=mybir.AluOpType.max,
                )
                nrmv = prologp.tile([P, 8], F32, tag="nrmv")
                nc.scalar.activation(
                    out=nrmv[:pp], in_=nsafe[:pp],
                    func=mybir.ActivationFunctionType.Sqrt,
                )
                rcp = prologp.tile([P, 8, 1], F32, tag="rcp")
                nc.vector.reciprocal(rcp[:pp, :, 0], nrmv[:pp])
                xnt = prologp.tile([P, 8, D], BF16, tag="xnt")
                nc.vector.tensor_tensor(
                    out=xnt[:pp], in0=xt[:pp],
                    in1=rcp[:pp].to_broadcast([pp, 8, D]),
                    op=mybir.AluOpType.mult,
                )
                nc.scalar.dma_start(out=xd_tab[r0 : r0 + rows, :], in_=xnt[:pp])
                r0 += rows


            cast_table(x_ext, xb_lo, n_lo)
            if N > hi_base:
                cast_table(x_ext[hi_base:N, :], xb_hi, N - hi_base)

            # ---- edge loop ----
            self_q = [0]
            for b in range(NB):
                idxt = idxp.tile([P, ga], I16, tag="idxt")
                nc.sync.dma_start(out=idxt[:], in_=idx_all[:, b * ga : (b + 1) * ga])
                sl = idxt[:, 0:gl]
                sh = idxt[:, gl : gl + gh]
                di = idxt[:, gl + gh : ga]
                dblk = idxp.tile([P, tpb, 1], BF16, tag="dblk")
                nc.scalar.dma_start(
                    out=dblk[:], in_=dst_blk[:, b * tpb : (b + 1) * tpb]
                )

                def gather_rows(out_tile, tab_ap, idx_tile, total):
                    # SWDGE descriptor-ring capacity caps one call at ~1024 rows;
                    # round-robin the 4 queues so all Q7 pairs generate descs
                    off = 0
                    while off < total:
                        ni = min(1024, total - off)
                        nc.gpsimd.dma_gather(
                            out_tile[:, off // P : (off + ni) // P, :],
                            tab_ap,
                            idx_tile[:, off // 16 : (off + ni) // 16],
                            ni, ni, D,
                            queue_num=self_q[0] % 4,
                        )
                        self_q[0] += 1
                        off += ni

                xs_lo = gslp.tile([P, tpb_lo, D], BF16, tag="xsl")
                gather_rows(xs_lo, xb_lo[:, :], sl, tpb_lo * P)
                xs_hi = gshp.tile([P, tpb_hi, D], BF16, tag="xsh")
                gather_rows(xs_hi, xb_hi[:, :], sh, tpb_hi * P)
                xd_blk = gdp.tile([P, tpb, D], BF16, tag="xd")
                gather_rows(xd_blk, xd_tab[:, :], di, tpb * P)

                # ---- block-wide logits: cos = dot * rsqrt(ss); |xd| == 1
                prod = workp.tile([P, tpb, D], BF16, tag="prod")
                ssA = colp.tile([P, tpb], BF16, tag="ssA")
                nc.scalar.activation(
                    out=prod[:, 0:tpb_lo, :], in_=xs_lo[:],
                    func=mybir.ActivationFunctionType.Square,
                )
                nc.scalar.activation(
                    out=prod[:, tpb_lo:tpb, :], in_=xs_hi[:],
                    func=mybir.ActivationFunctionType.Square,
                )
                with nc.allow_low_precision(reason="bf16 stats, tol 2e-2"):
                    nc.vector.tensor_reduce(
                        out=ssA[:], in_=prod[:], axis=mybir.AxisListType.X,
                        op=mybir.AluOpType.add,
                    )
                dotA = colp.tile([P, tpb], BF16, tag="dotA")
                nc.vector.tensor_tensor(
                    out=prod[:, 0:tpb_lo, :], in0=xs_lo[:],
                    in1=xd_blk[:, 0:tpb_lo, :], op=mybir.AluOpType.mult,
                )
                nc.vector.tensor_tensor(
                    out=prod[:, tpb_lo:tpb, :], in0=xs_hi[:],
                    in1=xd_blk[:, tpb_lo:tpb, :], op=mybir.AluOpType.mult,
                )
                with nc.allow_low_precision(reason="bf16 stats, tol 2e-2"):
                    nc.vector.tensor_reduce(
                        out=dotA[:], in_=prod[:], axis=mybir.AxisListType.X,
                        op=mybir.AluOpType.add,
                    )
                nrmA = colp.tile([P, tpb], F32, tag="nrmA")
                nc.scalar.activation(
                    out=nrmA[:], in_=ssA[:],
                    func=mybir.ActivationFunctionType.Sqrt,
                )
                rsA = colp.tile([P, tpb], F32, tag="rsA")
                nc.vector.reciprocal(rsA[:], nrmA[:])
                argA = colp.tile([P, tpb], F32, tag="argA")
                nc.vector.tensor_tensor(
                    out=argA[:], in0=dotA[:], in1=rsA[:], op=mybir.AluOpType.mult
                )
                wA = colp.tile([P, tpb, 1], BF16, tag="wA")
                nc.scalar.activation(
                    out=wA[:, :, 0], in_=argA[:],
                    func=mybir.ActivationFunctionType.Exp, scale=beta_sb[:],
                )
                # ---- block-wide one-hot * w
                pwA = pwp.tile([P, tpb, P], BF16, tag="pwA")
                nc.vector.tensor_tensor(
                    out=prod[:], in0=iota_bf[:][:, None, :].to_broadcast([P, tpb, P]),
                    in1=dblk[:].to_broadcast([P, tpb, P]),
                    op=mybir.AluOpType.is_equal,
                )
                nc.vector.tensor_tensor(
                    out=pwA[:], in0=prod[:],
                    in1=wA[:].to_broadcast([P, tpb, P]),
                    op=mybir.AluOpType.mult,
                )

                psum = psump.tile([P, D + 1], F32, tag="acc")

                for t in range(tpb):
                    if t < tpb_lo:
                        xs = xs_lo[:, t, :]
                    else:
                        xs = xs_hi[:, t - tpb_lo, :]
                    nc.tensor.matmul(
                        out=psum[:, 0:D], lhsT=pwA[:, t, :], rhs=xs,
                        start=(t == 0), stop=False, skip_group_check=True,
                    )
                    nc.tensor.matmul(
                        out=psum[:, D : D + 1], lhsT=pwA[:, t, :], rhs=ones_bf[:],
                        start=False, stop=(t == tpb - 1), skip_group_check=True,
                    )

                # epilogue: rows = relu(M / s); scatter rows to out
                s_safe = colp.tile([P, 1], F32, tag="ssafe")
                nc.vector.tensor_scalar(
                    out=s_safe[:], in0=psum[:, D : D + 1], scalar1=1e-30,
                    scalar2=None, op0=mybir.AluOpType.max,
                )
                sinv = colp.tile([P, 1], F32, tag="sinv")
                nc.vector.reciprocal(sinv[:], s_safe[:])
                orow = orowp.tile([P, D], F32, tag="orow")
                nc.scalar.activation(
                    out=orow[:], in_=psum[:, 0:D],
                    func=mybir.ActivationFunctionType.Relu, scale=sinv[:],
                )
                nc.sync.dma_start(
                    out=out_ext[b * P : (b + 1) * P, :], in_=orow[:]
                )

    nc.compile()
    return nc


def _wrap16(vals, ncols, pad):
    """[n] -> [128, ncols] int16 in dma_gather's 16-wrapped, 8x-replicated
    partition layout (idx j at [j%16, j//16])."""
    full = np.full(ncols * 16, pad, np.int64)
    full[: len(vals)] = vals
    w = full.reshape(ncols, 16).T.astype(np.int16)  # [16, ncols]
    return np.tile(w, (8, 1))


def _decompose(dst_sorted, src_sorted, N, tpb_lo, tpb_hi, hi_base, max_nodes=P):
    """Blocks of consecutive nodes with <=max_nodes nodes, <=tpb_lo*128
    low-src edges and <=tpb_hi*128 high-src edges."""
    deg = np.bincount(dst_sorted, minlength=N)
    deg_lo = np.bincount(dst_sorted[src_sorted < hi_base], minlength=N)
    deg_hi = deg - deg_lo
    cap_lo, cap_hi = tpb_lo * P, tpb_hi * P
    assert deg_lo.max() <= cap_lo and deg_hi.max() <= cap_hi
    blocks = []
    n0 = e0 = 0
    lo = hi = 0
    for node in range(N):
        dl, dh = int(deg_lo[node]), int(deg_hi[node])
        if (node - n0) >= max_nodes or lo + dl > cap_lo or hi + dh > cap_hi:
            blocks.append((n0, node, e0, e0 + lo + hi))
            n0, e0 = node, e0 + lo + hi
            lo = hi = 0
        lo += dl
        hi += dh
    blocks.append((n0, N, e0, e0 + lo + hi))
    return blocks


def _prep_inputs(x, beta, edge_index, N, D, tpb_lo, tpb_hi, hi_base, ncores):
    tpb = tpb_lo + tpb_hi
    gl, gh, gt = tpb_lo * 8, tpb_hi * 8, tpb * 8
    loop = np.arange(N, dtype=np.int64)
    src = np.concatenate([np.asarray(edge_index[0]), loop]).astype(np.int64)
    dst = np.concatenate([np.asarray(edge_index[1]), loop]).astype(np.int64)
    order = np.argsort(dst, kind="stable")
    src_s = src[order]
    dst_s = dst[order]

    blocks = _decompose(dst_s, src_s, N, tpb_lo, tpb_hi, hi_base)
    nbt = len(blocks)
    sizes = [nbt // ncores + (1 if i < nbt % ncores else 0) for i in range(ncores)]
    NB = max(sizes)

    core_blocks, bpos = [], 0
    for k in range(ncores):
        core_blocks.append(blocks[bpos : bpos + sizes[k]])
        bpos += sizes[k]
    ranges = [
        (blks[0][0], blks[-1][1]) if blks else (0, 0) for blks in core_blocks
    ]
    ndst = max(hi - lo for lo, hi in ranges)
    ndst = (ndst + 1023) // 1024 * 1024  # prologue supertile multiple
    assert ndst <= 32767

    xf32 = np.ascontiguousarray(np.asarray(x), dtype=np.float32)
    beta128 = np.full((P, 1), float(np.asarray(beta).reshape(-1)[0]), np.float32)

    in_maps = []
    for k in range(ncores):
        blks = core_blocks[k]
        lo_k = ranges[k][0]
        a_sl = np.zeros((P, NB * gl), np.int16)
        a_sh = np.zeros((P, NB * gh), np.int16)
        a_di = np.zeros((P, NB * gt), np.int16)
        a_blk = np.full((tpb * NB, P), BLK_SENTINEL, np.float32)
        for bi, (n0, n1, e0, e1) in enumerate(blks):
            s = src_s[e0:e1]
            d = dst_s[e0:e1]
            lomask = s < hi_base
            s_lo, d_lo = s[lomask], d[lomask]
            s_hi, d_hi = s[~lomask] - hi_base, d[~lomask]
            nlo, nhi = len(s_lo), len(s_hi)
            a_sl[:, bi * gl : (bi + 1) * gl] = _wrap16(s_lo, gl, 0)
            a_sh[:, bi * gh : (bi + 1) * gh] = _wrap16(s_hi, gh, 0)
            # combined order: lo edges in slots [0, tpb_lo*128), hi after
            dcomb = np.zeros(tpb * P, np.int64)
            dcomb[:nlo] = d_lo - lo_k
            dcomb[tpb_lo * P : tpb_lo * P + nhi] = d_hi - lo_k
            a_di[:, bi * gt : (bi + 1) * gt] = _wrap16(dcomb, gt, 0)
            bcomb = np.full(tpb * P, BLK_SENTINEL, np.float32)
            bcomb[:nlo] = (d_lo - n0).astype(np.float32)
            bcomb[tpb_lo * P : tpb_lo * P + nhi] = (d_hi - n0).astype(np.float32)
            a_blk[bi * tpb : (bi + 1) * tpb] = bcomb.reshape(tpb, P)
        xdst = np.zeros((ndst, D), np.float32)
        span = ranges[k][1] - lo_k
        xdst[:span] = xf32[lo_k : ranges[k][1]]
        a_idx = np.zeros((P, NB * (gl + gh + gt)), np.int16)
        ga = gl + gh + gt
        for bi in range(NB):
            a_idx[:, bi * ga : bi * ga + gl] = a_sl[:, bi * gl : (bi + 1) * gl]
            a_idx[:, bi * ga + gl : bi * ga + gl + gh] = a_sh[:, bi * gh : (bi + 1) * gh]
            a_idx[:, bi * ga + gl + gh : (bi + 1) * ga] = a_di[:, bi * gt : (bi + 1) * gt]
        in_maps.append(
            {
                "x": xf32,
                "xdst": xdst,
                "idx_all": a_idx,
                "dst_blk": np.ascontiguousarray(a_blk.T).astype(ml_dtypes.bfloat16),
                "beta128": beta128,
            }
        )
    return in_maps, ranges, NB, ndst, core_blocks


def _enable_axon_ntff():
    """Install the NTFF profile hook that the stub antenv package lacks."""
    import sys, types
    try:
        import antenv

        if "antenv.axon_hooks" not in sys.modules:
            mod = types.ModuleType("antenv.axon_hooks")
            mod._hook = None
            mod.set_axon_ntff_profile_hook = lambda h: setattr(mod, "_hook", h)
            mod.get_axon_ntff_profile_hook = lambda: mod._hook
            sys.modules["antenv.axon_hooks"] = mod
            antenv.axon_hooks = mod
            from trn_agent_boot.trn_boot import _ntff_profile_via_ctypes

            mod._hook = _ntff_profile_via_ctypes("/opt/axon/libaxon_pjrt.so")
        import concourse.bass_utils as bu

        bu.upload_artifacts = lambda tmpdir: tmpdir
        return True
    except Exception as e:
        print(f"ntff hook install failed: {e}")
        return False


def _run(x, beta, edge_index, trace=False):
    from concourse.bass_utils import run_bass_kernel_spmd

    if trace:
        trace = _enable_axon_ntff()
    N, D = x.shape
    in_maps, ranges, NB, ndst, core_blocks = _prep_inputs(
        x, beta, edge_index, N, D, TPB_LO, TPB_HI, HI_BASE, NCORES
    )
    nc = _build_graph(N, D, NB, TPB_LO, TPB_HI, ndst, HI_BASE)
    res = run_bass_kernel_spmd(
        nc, in_maps, core_ids=list(range(NCORES)), trace=trace
    )
    out = np.zeros((N, D), np.float32)
    for k in range(NCORES):
        co = res.results[k]["out"]
        for bi, (n0, n1, e0, e1) in enumerate(core_blocks[k]):
            out[n0:n1] = co[bi * P : bi * P + (n1 - n0)]
    return out, res


def kernel(x, beta, edge_index):
    out, _ = _run(
        np.asarray(x), np.asarray(beta), np.asarray(edge_index), trace=False
    )
    return out


# revision 27
# speedup vs baseline: 1.0067x; 1.0067x over previous
"""AGNN layer (cosine-attention message passing) on 8 TRN2 NeuronCores.

Host sharding: append self-loops, sort edges by destination node, cut the node
range into blocks (<=128 nodes, bounded edge count), hand contiguous block
runs to the 8 cores. Every softmax segment then lives on one core: no
collectives anywhere.

Device kernel per core (identical SPMD graph, per-core data):
  prologue: cast x (f32) -> bf16 src gather table; normalize the core's own
            node-range slice (xdst input) -> unit-norm bf16 dst table.
  per block: dma_gather (4 SWDGE queues round-robin, <=1024 rows/call)
            fetches per-edge rows: src rows from the low half [0,32768) and
            high half [32768,N) of the table (int16 index limit forces the
            static lo/hi tile split), dst rows from the per-core dst table
            (relative indices). Block-wide DVE/ACT ops compute per-edge
            cos = (x_s . xn_d) * rsqrt(|x_s|^2) and w = exp(beta*cos), and
            build the one-hot(dst)*w matrix via iota/is_equal; per 128-edge
            tile, two PSUM-accumulating matmuls build per-node sum(w*x_src)
            and sum(w).
  per block epilogue: rows = relu(M/s) written to a compact per-block
            output; the host scatters block rows back to node order.

Logits are cosines scaled by beta (bounded), so exp never overflows and the
reference's segment-max subtraction cancels exactly -- single pass suffices.
"""

import numpy as np
import ml_dtypes

import concourse.bacc as bacc
import concourse.mybir as mybir
import concourse.tile as tile
from concourse.library_config import mlp as _mlp_lib
import concourse.tile_sem_assignment as _tsa

# Tile's DMASW-lane round-robin is SWDGE-queue-oblivious; a completion sem
# shared by two queues desyncs the ucode's per-queue ring-reclaim waits.
# Partition the 8 lanes so queue q owns lanes {2q, 2q+1}.
_orig_assign_tick = _tsa.TileClockTick._assign_tick


def _assign_tick_queue_aware(self, inst):
    if (
        isinstance(inst, mybir.InstDMAGatherAnt)
        and inst.engine == mybir.EngineType.Pool
    ):
        qn = inst.queue_num
        if not hasattr(self, "_qcnt"):
            self._qcnt = {}
        cnt = self._qcnt.get(qn, 0)
        self._qcnt[qn] = cnt + 1
        self.next_sw_dma_idx = qn * 2 + (cnt % 2)
    return _orig_assign_tick(self, inst)


_tsa.TileClockTick._assign_tick = _assign_tick_queue_aware

P = 128
N_NODES = 50000
D_FEAT = 128
NCORES = 8
HI_BASE = 32768  # int16 gather index limit
TPB_LO = 23  # tiles per block for src<HI_BASE edges
TPB_HI = 12  # tiles per block for src>=HI_BASE edges
BLK_SENTINEL = 300.0

F32 = mybir.dt.float32
BF16 = mybir.dt.bfloat16
I16 = mybir.dt.int16


def _build_graph(N, D, NB, tpb_lo, tpb_hi, ndst, hi_base):
    tpb = tpb_lo + tpb_hi
    gl, gh, gt = tpb_lo * 8, tpb_hi * 8, tpb * 8  # idx cols (16-wrapped)
    nc = bacc.Bacc(
        "TRN2", target_bir_lowering=False, debug=False, enable_asserts=False,
        num_swdge_queues=4,
    )
    x_ext = nc.dram_tensor("x", [N, D], F32, kind="ExternalInput").ap()
    xdst_ext = nc.dram_tensor("xdst", [ndst, D], F32, kind="ExternalInput").ap()
    ga = gl + gh + gt
    idx_all = nc.dram_tensor("idx_all", [P, NB * ga], I16, kind="ExternalInput").ap()
    dst_blk = nc.dram_tensor("dst_blk", [P, NB * tpb], BF16, kind="ExternalInput").ap()
    beta128 = nc.dram_tensor("beta128", [P, 1], F32, kind="ExternalInput").ap()
    out_ext = nc.dram_tensor("out", [NB * P, D], F32, kind="ExternalOutput").ap()

    n_lo = min(hi_base, N)
    n_hi = max(N - hi_base, 8)
    xb_lo = nc.dram_tensor("xb_lo", [n_lo, D], BF16).ap()
    xb_hi = nc.dram_tensor("xb_hi", [n_hi, D], BF16).ap()
    xd_tab = nc.dram_tensor("xd_table", [ndst, D], BF16).ap()

    with tile.TileContext(nc) as tc:
        with (
            tc.tile_pool(name="const", bufs=1) as constp,
            tc.tile_pool(name="prolog", bufs=3) as prologp,
            tc.tile_pool(name="idx", bufs=3) as idxp,
            tc.tile_pool(name="gsl", bufs=3) as gslp,
            tc.tile_pool(name="gsh", bufs=3) as gshp,
            tc.tile_pool(name="gd", bufs=3) as gdp,
            tc.tile_pool(name="work", bufs=2) as workp,
            tc.tile_pool(name="pw", bufs=3) as pwp,
            tc.tile_pool(name="cols", bufs=4) as colp,
            tc.tile_pool(name="orow", bufs=2) as orowp,
            tc.tile_pool(name="psum", bufs=4, space="PSUM") as psump,
        ):
            nc.gpsimd.load_library(_mlp_lib)

            # ---- constants ----
            iota_i16 = constp.tile([P, P], I16)
            nc.gpsimd.iota(iota_i16[:], pattern=[[1, P]], base=0, channel_multiplier=0)
            iota_bf = constp.tile([P, P], BF16)
            nc.vector.tensor_copy(iota_bf[:], iota_i16[:])
            ones_bf = constp.tile([P, 1], BF16)
            nc.vector.memset(ones_bf[:], 1.0)
            beta_sb = constp.tile([P, 1], F32)
            nc.sync.dma_start(out=beta_sb[:], in_=beta128[:, :])

            # ---- prologue: cast x -> bf16 tables ----
            def cast_table(src_ap, dst_ap, nrows):
                r0 = 0
                while r0 < nrows:
                    rows = min(2048, nrows - r0)
                    rpp = 16
                    while rows % rpp:
                        rpp //= 2
                    pp = rows // rpp
                    xt = prologp.tile([P, 16, D], F32, tag="xt")
                    nc.sync.dma_start(
                        out=xt[:pp, 0:rpp, :], in_=src_ap[r0 : r0 + rows, :]
                    )
                    xbt = prologp.tile([P, 16, D], BF16, tag="xbt")
                    nc.vector.tensor_copy(xbt[:pp, 0:rpp, :], xt[:pp, 0:rpp, :])
                    nc.scalar.dma_start(
                        out=dst_ap[r0 : r0 + rows, :], in_=xbt[:pp, 0:rpp, :]
                    )
                    r0 += rows

            cast_table(x_ext, xb_lo, n_lo)
            if N > hi_base:
                cast_table(x_ext[hi_base:N, :], xb_hi, N - hi_base)

            # dst table is pre-normalized: xd_tab rows = x / |x|
            r0 = 0
            while r0 < ndst:
                rows = min(1024, ndst - r0)
                pp = rows // 8
                xt = prologp.tile([P, 8, D], F32, tag="xt")
                nc.sync.dma_start(out=xt[:pp], in_=xdst_ext[r0 : r0 + rows, :])
                sqv = prologp.tile([P, 8, D], F32, tag="sqv")
                nc.vector.tensor_tensor(
                    out=sqv[:pp], in0=xt[:pp], in1=xt[:pp], op=mybir.AluOpType.mult
                )
                ssum = prologp.tile([P, 8], F32, tag="ssum")
                nc.vector.tensor_reduce(
                    out=ssum[:pp], in_=sqv[:pp], axis=mybir.AxisListType.X,
                    op=mybir.AluOpType.add,
                )
                # rows of zeros (tail padding) -> clamp to avoid inf
                nsafe = prologp.tile([P, 8], F32, tag="nsafe")
                nc.vector.tensor_scalar(
                    out=nsafe[:pp], in0=ssum[:pp], scalar1=1e-30, scalar2=None,
                    op0=mybir.AluOpType.max,
                )
                nrmv = prologp.tile([P, 8], F32, tag="nrmv")
                nc.scalar.activation(
                    out=nrmv[:pp], in_=nsafe[:pp],
                    func=mybir.ActivationFunctionType.Sqrt,
                )
                rcp = prologp.tile([P, 8, 1], F32, tag="rcp")
                nc.vector.reciprocal(rcp[:pp, :, 0], nrmv[:pp])
                xnt = prologp.tile([P, 8, D], BF16, tag="xnt")
                nc.vector.tensor_tensor(
                    out=xnt[:pp], in0=xt[:pp],
                    in1=rcp[:pp].to_broadcast([pp, 8, D]),
                    op=mybir.AluOpType.mult,
                )
                nc.scalar.dma_start(out=xd_tab[r0 : r0 + rows, :], in_=xnt[:pp])
                r0 += rows

            # ---- edge loop ----
            self_q = [0]
            for b in range(NB):
                idxt = idxp.tile([P, ga], I16, tag="idxt")
                nc.sync.dma_start(out=idxt[:], in_=idx_all[:, b * ga : (b + 1) * ga])
                sl = idxt[:, 0:gl]
                sh = idxt[:, gl : gl + gh]
                di = idxt[:, gl + gh : ga]
                dblk = idxp.tile([P, tpb, 1], BF16, tag="dblk")
                nc.scalar.dma_start(
                    out=dblk[:], in_=dst_blk[:, b * tpb : (b + 1) * tpb]
                )

                def gather_rows(out_tile, tab_ap, idx_tile, total):
                    # SWDGE descriptor-ring capacity caps one call at ~1024 rows;
                    # round-robin the 4 queues so all Q7 pairs generate descs
                    off = 0
                    while off < total:
                        ni = min(1024, total - off)
                        nc.gpsimd.dma_gather(
                            out_tile[:, off // P : (off + ni) // P, :],
                            tab_ap,
                            idx_tile[:, off // 16 : (off + ni) // 16],
                            ni, ni, D,
                            queue_num=self_q[0] % 4,
                        )
                        self_q[0] += 1
                        off += ni

                xs_lo = gslp.tile([P, tpb_lo, D], BF16, tag="xsl")
                gather_rows(xs_lo, xb_lo[:, :], sl, tpb_lo * P)
                xs_hi = gshp.tile([P, tpb_hi, D], BF16, tag="xsh")
                gather_rows(xs_hi, xb_hi[:, :], sh, tpb_hi * P)
                xd_blk = gdp.tile([P, tpb, D], BF16, tag="xd")
                gather_rows(xd_blk, xd_tab[:, :], di, tpb * P)

                # ---- block-wide logits: cos = dot * rsqrt(ss); |xd| == 1
                prod = workp.tile([P, tpb, D], BF16, tag="prod")
                ssA = colp.tile([P, tpb], BF16, tag="ssA")
                nc.scalar.activation(
                    out=prod[:, 0:tpb_lo, :], in_=xs_lo[:],
                    func=mybir.ActivationFunctionType.Square,
                )
                nc.scalar.activation(
                    out=prod[:, tpb_lo:tpb, :], in_=xs_hi[:],
                    func=mybir.ActivationFunctionType.Square,
                )
                with nc.allow_low_precision(reason="bf16 stats, tol 2e-2"):
                    nc.vector.tensor_reduce(
                        out=ssA[:], in_=prod[:], axis=mybir.AxisListType.X,
                        op=mybir.AluOpType.add,
                    )
                dotA = colp.tile([P, tpb], BF16, tag="dotA")
                nc.vector.tensor_tensor(
                    out=prod[:, 0:tpb_lo, :], in0=xs_lo[:],
                    in1=xd_blk[:, 0:tpb_lo, :], op=mybir.AluOpType.mult,
                )
                nc.vector.tensor_tensor(
                    out=prod[:, tpb_lo:tpb, :], in0=xs_hi[:],
                    in1=xd_blk[:, tpb_lo:tpb, :], op=mybir.AluOpType.mult,
                )
                with nc.allow_low_precision(reason="bf16 stats, tol 2e-2"):
                    nc.vector.tensor_reduce(
                        out=dotA[:], in_=prod[:], axis=mybir.AxisListType.X,
                        op=mybir.AluOpType.add,
                    )
                nrmA = colp.tile([P, tpb], F32, tag="nrmA")
                nc.scalar.activation(
                    out=nrmA[:], in_=ssA[:],
                    func=mybir.ActivationFunctionType.Sqrt,
                )
                rsA = colp.tile([P, tpb], F32, tag="rsA")
                nc.vector.reciprocal(rsA[:], nrmA[:])
                argA = colp.tile([P, tpb], F32, tag="argA")
                nc.vector.tensor_tensor(
                    out=argA[:], in0=dotA[:], in1=rsA[:], op=mybir.AluOpType.mult
                )
                wA = colp.tile([P, tpb, 1], BF16, tag="wA")
                nc.scalar.activation(
                    out=wA[:, :, 0], in_=argA[:],
                    func=mybir.ActivationFunctionType.Exp, scale=beta_sb[:],
                )
                # ---- block-wide one-hot * w
                pwA = pwp.tile([P, tpb, P], BF16, tag="pwA")
                nc.vector.tensor_tensor(
                    out=prod[:], in0=iota_bf[:][:, None, :].to_broadcast([P, tpb, P]),
                    in1=dblk[:].to_broadcast([P, tpb, P]),
                    op=mybir.AluOpType.is_equal,
                )
                nc.vector.tensor_tensor(
                    out=pwA[:], in0=prod[:],
                    in1=wA[:].to_broadcast([P, tpb, P]),
                    op=mybir.AluOpType.mult,
                )

                psum = psump.tile([P, D + 1], F32, tag="acc")

                for t in range(tpb):
                    if t < tpb_lo:
                        xs = xs_lo[:, t, :]
                    else:
                        xs = xs_hi[:, t - tpb_lo, :]
                    nc.tensor.matmul(
                        out=psum[:, 0:D], lhsT=pwA[:, t, :], rhs=xs,
                        start=(t == 0), stop=False, skip_group_check=True,
                    )
                    nc.tensor.matmul(
                        out=psum[:, D : D + 1], lhsT=pwA[:, t, :], rhs=ones_bf[:],
                        start=False, stop=(t == tpb - 1), skip_group_check=True,
                    )

                # epilogue: rows = relu(M / s); scatter rows to out
                s_safe = colp.tile([P, 1], F32, tag="ssafe")
                nc.vector.tensor_scalar(
                    out=s_safe[:], in0=psum[:, D : D + 1], scalar1=1e-30,
                    scalar2=None, op0=mybir.AluOpType.max,
                )
                sinv = colp.tile([P, 1], F32, tag="sinv")
                nc.vector.reciprocal(sinv[:], s_safe[:])
                orow = orowp.tile([P, D], F32, tag="orow")
                nc.vector.tensor_scalar(
                    out=orow[:], in0=psum[:, 0:D], scalar1=sinv[:], scalar2=0.0,
                    op0=mybir.AluOpType.mult, op1=mybir.AluOpType.max,
                )
                nc.sync.dma_start(
                    out=out_ext[b * P : (b + 1) * P, :], in_=orow[:]
                )

    nc.compile()
    return nc


def _wrap16(vals, ncols, pad):
    """[n] -> [128, ncols] int16 in dma_gather's 16-wrapped, 8x-replicated
    partition layout (idx j at [j%16, j//16])."""
    full = np.full(ncols * 16, pad, np.int64)
    full[: len(vals)] = vals
    w = full.reshape(ncols, 16).T.astype(np.int16)  # [16, ncols]
    return np.tile(w, (8, 1))


def _decompose(dst_sorted, src_sorted, N, tpb_lo, tpb_hi, hi_base, max_nodes=P):
    """Blocks of consecutive nodes with <=max_nodes nodes, <=tpb_lo*128
    low-src edges and <=tpb_hi*128 high-src edges."""
    deg = np.bincount(dst_sorted, minlength=N)
    deg_lo = np.bincount(dst_sorted[src_sorted < hi_base], minlength=N)
    deg_hi = deg - deg_lo
    cap_lo, cap_hi = tpb_lo * P, tpb_hi * P
    assert deg_lo.max() <= cap_lo and deg_hi.max() <= cap_hi
    blocks = []
    n0 = e0 = 0
    lo = hi = 0
    for node in range(N):
        dl, dh = int(deg_lo[node]), int(deg_hi[node])
        if (node - n0) >= max_nodes or lo + dl > cap_lo or hi + dh > cap_hi:
            blocks.append((n0, node, e0, e0 + lo + hi))
            n0, e0 = node, e0 + lo + hi
            lo = hi = 0
        lo += dl
        hi += dh
    blocks.append((n0, N, e0, e0 + lo + hi))
    return blocks


def _prep_inputs(x, beta, edge_index, N, D, tpb_lo, tpb_hi, hi_base, ncores):
    tpb = tpb_lo + tpb_hi
    gl, gh, gt = tpb_lo * 8, tpb_hi * 8, tpb * 8
    loop = np.arange(N, dtype=np.int64)
    src = np.concatenate([np.asarray(edge_index[0]), loop]).astype(np.int64)
    dst = np.concatenate([np.asarray(edge_index[1]), loop]).astype(np.int64)
    order = np.argsort(dst, kind="stable")
    src_s = src[order]
    dst_s = dst[order]

    blocks = _decompose(dst_s, src_s, N, tpb_lo, tpb_hi, hi_base)
    nbt = len(blocks)
    sizes = [nbt // ncores + (1 if i < nbt % ncores else 0) for i in range(ncores)]
    NB = max(sizes)

    core_blocks, bpos = [], 0
    for k in range(ncores):
        core_blocks.append(blocks[bpos : bpos + sizes[k]])
        bpos += sizes[k]
    ranges = [
        (blks[0][0], blks[-1][1]) if blks else (0, 0) for blks in core_blocks
    ]
    ndst = max(hi - lo for lo, hi in ranges)
    ndst = (ndst + 1023) // 1024 * 1024  # prologue supertile multiple
    assert ndst <= 32767

    xf32 = np.ascontiguousarray(np.asarray(x), dtype=np.float32)
    beta128 = np.full((P, 1), float(np.asarray(beta).reshape(-1)[0]), np.float32)

    in_maps = []
    for k in range(ncores):
        blks = core_blocks[k]
        lo_k = ranges[k][0]
        a_sl = np.zeros((P, NB * gl), np.int16)
        a_sh = np.zeros((P, NB * gh), np.int16)
        a_di = np.zeros((P, NB * gt), np.int16)
        a_blk = np.full((tpb * NB, P), BLK_SENTINEL, np.float32)
        for bi, (n0, n1, e0, e1) in enumerate(blks):
            s = src_s[e0:e1]
            d = dst_s[e0:e1]
            lomask = s < hi_base
            s_lo, d_lo = s[lomask], d[lomask]
            s_hi, d_hi = s[~lomask] - hi_base, d[~lomask]
            nlo, nhi = len(s_lo), len(s_hi)
            a_sl[:, bi * gl : (bi + 1) * gl] = _wrap16(s_lo, gl, 0)
            a_sh[:, bi * gh : (bi + 1) * gh] = _wrap16(s_hi, gh, 0)
            # combined order: lo edges in slots [0, tpb_lo*128), hi after
            dcomb = np.zeros(tpb * P, np.int64)
            dcomb[:nlo] = d_lo - lo_k
            dcomb[tpb_lo * P : tpb_lo * P + nhi] = d_hi - lo_k
            a_di[:, bi * gt : (bi + 1) * gt] = _wrap16(dcomb, gt, 0)
            bcomb = np.full(tpb * P, BLK_SENTINEL, np.float32)
            bcomb[:nlo] = (d_lo - n0).astype(np.float32)
            bcomb[tpb_lo * P : tpb_lo * P + nhi] = (d_hi - n0).astype(np.float32)
            a_blk[bi * tpb : (bi + 1) * tpb] = bcomb.reshape(tpb, P)
        xdst = np.zeros((ndst, D), np.float32)
        span = ranges[k][1] - lo_k
        xdst[:span] = xf32[lo_k : ranges[k][1]]
        a_idx = np.zeros((P, NB * (gl + gh + gt)), np.int16)
        ga = gl + gh + gt
        for bi in range(NB):
            a_idx[:, bi * ga : bi * ga + gl] = a_sl[:, bi * gl : (bi + 1) * gl]
            a_idx[:, bi * ga + gl : bi * ga + gl + gh] = a_sh[:, bi * gh : (bi + 1) * gh]
            a_idx[:, bi * ga + gl + gh : (bi + 1) * ga] = a_di[:, bi * gt : (bi + 1) * gt]
        in_maps.append(
            {
                "x": xf32,
                "xdst": xdst,
                "idx_all": a_idx,
                "dst_blk": np.ascontiguousarray(a_blk.T).astype(ml_dtypes.bfloat16),
                "beta128": beta128,
            }
        )
    return in_maps, ranges, NB, ndst, core_blocks


def _enable_axon_ntff():
    """Install the NTFF profile hook that the stub antenv package lacks."""
    import sys, types
    try:
        import antenv

        if "antenv.axon_hooks" not in sys.modules:
            mod = types.ModuleType("antenv.axon_hooks")
            mod._hook = None
            mod.set_axon_ntff_profile_hook = lambda h: setattr(mod, "_hook", h)
            mod.get_axon_ntff_profile_hook = lambda: mod._hook
            sys.modules["antenv.axon_hooks"] = mod
            antenv.axon_hooks = mod
            from trn_agent_boot.trn_boot import _ntff_profile_via_ctypes

            mod._hook = _ntff_profile_via_ctypes("/opt/axon/libaxon_pjrt.so")
        import concourse.bass_utils as bu

        bu.upload_artifacts = lambda tmpdir: tmpdir
        return True
    except Exception as e:
        print(f"ntff hook install failed: {e}")
        return False


def _run(x, beta, edge_index, trace=False):
    from concourse.bass_utils import run_bass_kernel_spmd

    if trace:
        trace = _enable_axon_ntff()
    N, D = x.shape
    in_maps, ranges, NB, ndst, core_blocks = _prep_inputs(
        x, beta, edge_index, N, D, TPB_LO, TPB_HI, HI_BASE, NCORES
    )
    nc = _build_graph(N, D, NB, TPB_LO, TPB_HI, ndst, HI_BASE)
    res = run_bass_kernel_spmd(
        nc, in_maps, core_ids=list(range(NCORES)), trace=trace
    )
    out = np.zeros((N, D), np.float32)
    for k in range(NCORES):
        co = res.results[k]["out"]
        for bi, (n0, n1, e0, e1) in enumerate(core_blocks[k]):
            out[n0:n1] = co[bi * P : bi * P + (n1 - n0)]
    return out, res


def kernel(x, beta, edge_index):
    out, _ = _run(
        np.asarray(x), np.asarray(beta), np.asarray(edge_index), trace=False
    )
    return out


# revision 29
# speedup vs baseline: 1.1849x; 1.1770x over previous
"""AGNN layer (cosine-attention message passing) on 8 TRN2 NeuronCores.

Host sharding: append self-loops, sort edges by destination node, cut the node
range into blocks (<=128 nodes, bounded edge count), hand contiguous block
runs to the 8 cores. Every softmax segment then lives on one core: no
collectives anywhere.

Device kernel per core (identical SPMD graph, per-core data):
  prologue: cast x (f32) -> bf16 src gather table; normalize the core's own
            node-range slice (xdst input) -> unit-norm bf16 dst table.
  per block: dma_gather (4 SWDGE queues round-robin, <=1024 rows/call)
            fetches per-edge rows: src rows from the low half [0,32768) and
            high half [32768,N) of the table (int16 index limit forces the
            static lo/hi tile split), dst rows from the per-core dst table
            (relative indices). Block-wide DVE/ACT ops compute per-edge
            cos = (x_s . xn_d) * rsqrt(|x_s|^2) and w = exp(beta*cos), and
            build the one-hot(dst)*w matrix via iota/is_equal; per 128-edge
            tile, two PSUM-accumulating matmuls build per-node sum(w*x_src)
            and sum(w).
  per block epilogue: rows = relu(M/s) written to a compact per-block
            output; the host scatters block rows back to node order.

Logits are cosines scaled by beta (bounded), so exp never overflows and the
reference's segment-max subtraction cancels exactly -- single pass suffices.
"""

import numpy as np
import ml_dtypes

import concourse.bacc as bacc
import concourse.mybir as mybir
import concourse.tile as tile
from concourse.library_config import mlp as _mlp_lib
import concourse.tile_sem_assignment as _tsa

# Tile's DMASW-lane round-robin is SWDGE-queue-oblivious; a completion sem
# shared by two queues desyncs the ucode's per-queue ring-reclaim waits.
# Partition the 8 lanes so queue q owns lanes {2q, 2q+1}.
_orig_assign_tick = _tsa.TileClockTick._assign_tick


def _assign_tick_queue_aware(self, inst):
    if (
        isinstance(inst, mybir.InstDMAGatherAnt)
        and inst.engine == mybir.EngineType.Pool
    ):
        qn = inst.queue_num
        if not hasattr(self, "_qcnt"):
            self._qcnt = {}
        cnt = self._qcnt.get(qn, 0)
        self._qcnt[qn] = cnt + 1
        self.next_sw_dma_idx = qn * 2 + (cnt % 2)
    return _orig_assign_tick(self, inst)


_tsa.TileClockTick._assign_tick = _assign_tick_queue_aware

P = 128
N_NODES = 50000
D_FEAT = 128
NCORES = 8
HI_BASE = 32768  # int16 gather index limit
TPB_LO = 23  # tiles per block for src<HI_BASE edges
TPB_HI = 12  # tiles per block for src>=HI_BASE edges
BLK_SENTINEL = 300.0

F32 = mybir.dt.float32
BF16 = mybir.dt.bfloat16
I16 = mybir.dt.int16


def _build_graph(N, D, NB, tpb_lo, tpb_hi, ndst, hi_base):
    tpb = tpb_lo + tpb_hi
    gl, gh, gt = tpb_lo * 8, tpb_hi * 8, tpb * 8  # idx cols (16-wrapped)
    nc = bacc.Bacc(
        "TRN2", target_bir_lowering=False, debug=False, enable_asserts=False,
        num_swdge_queues=4,
    )
    x_ext = nc.dram_tensor("x", [N, D], F32, kind="ExternalInput").ap()
    xdst_ext = nc.dram_tensor("xdst", [ndst, D], F32, kind="ExternalInput").ap()
    ga = gl + gh + gt
    idx_all = nc.dram_tensor("idx_all", [P, NB * ga], I16, kind="ExternalInput").ap()
    dst_blk = nc.dram_tensor("dst_blk", [P, NB * tpb], BF16, kind="ExternalInput").ap()
    beta128 = nc.dram_tensor("beta128", [P, 1], F32, kind="ExternalInput").ap()
    out_ext = nc.dram_tensor("out", [NB * P, D], F32, kind="ExternalOutput").ap()

    n_lo = min(hi_base, N)
    n_hi = max(N - hi_base, 8)
    xb_lo = nc.dram_tensor("xb_lo", [n_lo, D], BF16).ap()
    xb_hi = nc.dram_tensor("xb_hi", [n_hi, D], BF16).ap()
    xd_tab = nc.dram_tensor("xd_table", [ndst, D], BF16).ap()

    with tile.TileContext(nc) as tc:
        with (
            tc.tile_pool(name="const", bufs=1) as constp,
            tc.tile_pool(name="prolog", bufs=3) as prologp,
            tc.tile_pool(name="idx", bufs=3) as idxp,
            tc.tile_pool(name="gsl", bufs=3) as gslp,
            tc.tile_pool(name="gsh", bufs=3) as gshp,
            tc.tile_pool(name="gd", bufs=3) as gdp,
            tc.tile_pool(name="work", bufs=2) as workp,
            tc.tile_pool(name="pw", bufs=3) as pwp,
            tc.tile_pool(name="cols", bufs=4) as colp,
            tc.tile_pool(name="orow", bufs=2) as orowp,
            tc.tile_pool(name="psum", bufs=4, space="PSUM") as psump,
        ):
            nc.gpsimd.load_library(_mlp_lib)

            # ---- constants ----
            iota_i16 = constp.tile([P, P], I16)
            nc.gpsimd.iota(iota_i16[:], pattern=[[1, P]], base=0, channel_multiplier=0)
            iota_bf = constp.tile([P, P], BF16)
            nc.vector.tensor_copy(iota_bf[:], iota_i16[:])
            ones_bf = constp.tile([P, 1], BF16)
            nc.vector.memset(ones_bf[:], 1.0)
            beta_sb = constp.tile([P, 1], F32)
            nc.sync.dma_start(out=beta_sb[:], in_=beta128[:, :])

            # ---- prologue: cast x -> bf16 tables ----
            def cast_table(src_ap, dst_ap, nrows):
                r0 = 0
                while r0 < nrows:
                    rows = min(2048, nrows - r0)
                    rpp = 16
                    while rows % rpp:
                        rpp //= 2
                    pp = rows // rpp
                    xt = prologp.tile([P, 16, D], F32, tag="xt")
                    nc.sync.dma_start(
                        out=xt[:pp, 0:rpp, :], in_=src_ap[r0 : r0 + rows, :]
                    )
                    xbt = prologp.tile([P, 16, D], BF16, tag="xbt")
                    nc.vector.tensor_copy(xbt[:pp, 0:rpp, :], xt[:pp, 0:rpp, :])
                    nc.scalar.dma_start(
                        out=dst_ap[r0 : r0 + rows, :], in_=xbt[:pp, 0:rpp, :]
                    )
                    r0 += rows

            cast_table(x_ext, xb_lo, n_lo)
            if N > hi_base:
                cast_table(x_ext[hi_base:N, :], xb_hi, N - hi_base)

            # dst table is pre-normalized: xd_tab rows = x / |x|
            r0 = 0
            while r0 < ndst:
                rows = min(1024, ndst - r0)
                pp = rows // 8
                xt = prologp.tile([P, 8, D], F32, tag="xt")
                nc.sync.dma_start(out=xt[:pp], in_=xdst_ext[r0 : r0 + rows, :])
                sqv = prologp.tile([P, 8, D], F32, tag="sqv")
                nc.vector.tensor_tensor(
                    out=sqv[:pp], in0=xt[:pp], in1=xt[:pp], op=mybir.AluOpType.mult
                )
                ssum = prologp.tile([P, 8], F32, tag="ssum")
                nc.vector.tensor_reduce(
                    out=ssum[:pp], in_=sqv[:pp], axis=mybir.AxisListType.X,
                    op=mybir.AluOpType.add,
                )
                # rows of zeros (tail padding) -> clamp to avoid inf
                nsafe = prologp.tile([P, 8], F32, tag="nsafe")
                nc.vector.tensor_scalar(
                    out=nsafe[:pp], in0=ssum[:pp], scalar1=1e-30, scalar2=None,
                    op0=mybir.AluOpType.max,
                )
                nrmv = prologp.tile([P, 8], F32, tag="nrmv")
                nc.scalar.activation(
                    out=nrmv[:pp], in_=nsafe[:pp],
                    func=mybir.ActivationFunctionType.Sqrt,
                )
                rcp = prologp.tile([P, 8, 1], F32, tag="rcp")
                nc.vector.reciprocal(rcp[:pp, :, 0], nrmv[:pp])
                xnt = prologp.tile([P, 8, D], BF16, tag="xnt")
                nc.vector.tensor_tensor(
                    out=xnt[:pp], in0=xt[:pp],
                    in1=rcp[:pp].to_broadcast([pp, 8, D]),
                    op=mybir.AluOpType.mult,
                )
                nc.scalar.dma_start(out=xd_tab[r0 : r0 + rows, :], in_=xnt[:pp])
                r0 += rows

            # ---- edge loop ----
            self_q = [0]
            for b in range(NB):
                idxt = idxp.tile([P, ga], I16, tag="idxt")
                nc.sync.dma_start(out=idxt[:], in_=idx_all[:, b * ga : (b + 1) * ga])
                sl = idxt[:, 0:gl]
                sh = idxt[:, gl : gl + gh]
                di = idxt[:, gl + gh : ga]
                dblk = idxp.tile([P, tpb, 1], BF16, tag="dblk")
                nc.sync.dma_start(
                    out=dblk[:], in_=dst_blk[:, b * tpb : (b + 1) * tpb]
                )

                def gather_rows(out_tile, tab_ap, idx_tile, total):
                    # SWDGE descriptor-ring capacity caps one call at ~1024 rows;
                    # round-robin the 4 queues so all Q7 pairs generate descs
                    off = 0
                    while off < total:
                        ni = min(1024, total - off)
                        nc.gpsimd.dma_gather(
                            out_tile[:, off // P : (off + ni) // P, :],
                            tab_ap,
                            idx_tile[:, off // 16 : (off + ni) // 16],
                            ni, ni, D,
                            queue_num=self_q[0] % 4,
                        )
                        self_q[0] += 1
                        off += ni

                xs_lo = gslp.tile([P, tpb_lo, D], BF16, tag="xsl")
                gather_rows(xs_lo, xb_lo[:, :], sl, tpb_lo * P)
                xs_hi = gshp.tile([P, tpb_hi, D], BF16, tag="xsh")
                gather_rows(xs_hi, xb_hi[:, :], sh, tpb_hi * P)
                xd_blk = gdp.tile([P, tpb, D], BF16, tag="xd")
                gather_rows(xd_blk, xd_tab[:, :], di, tpb * P)

                # ---- block-wide logits: cos = dot * rsqrt(ss); |xd| == 1
                prod = workp.tile([P, tpb, D], BF16, tag="prod")
                ssA = colp.tile([P, tpb], BF16, tag="ssA")
                nc.scalar.activation(
                    out=prod[:, 0:tpb_lo, :], in_=xs_lo[:],
                    func=mybir.ActivationFunctionType.Square,
                )
                nc.scalar.activation(
                    out=prod[:, tpb_lo:tpb, :], in_=xs_hi[:],
                    func=mybir.ActivationFunctionType.Square,
                )
                with nc.allow_low_precision(reason="bf16 stats, tol 2e-2"):
                    nc.vector.tensor_reduce(
                        out=ssA[:], in_=prod[:], axis=mybir.AxisListType.X,
                        op=mybir.AluOpType.add,
                    )
                dotA = colp.tile([P, tpb], BF16, tag="dotA")
                nc.vector.tensor_tensor(
                    out=prod[:, 0:tpb_lo, :], in0=xs_lo[:],
                    in1=xd_blk[:, 0:tpb_lo, :], op=mybir.AluOpType.mult,
                )
                nc.vector.tensor_tensor(
                    out=prod[:, tpb_lo:tpb, :], in0=xs_hi[:],
                    in1=xd_blk[:, tpb_lo:tpb, :], op=mybir.AluOpType.mult,
                )
                with nc.allow_low_precision(reason="bf16 stats, tol 2e-2"):
                    nc.vector.tensor_reduce(
                        out=dotA[:], in_=prod[:], axis=mybir.AxisListType.X,
                        op=mybir.AluOpType.add,
                    )
                nrmA = colp.tile([P, tpb], F32, tag="nrmA")
                nc.scalar.activation(
                    out=nrmA[:], in_=ssA[:],
                    func=mybir.ActivationFunctionType.Sqrt,
                )
                rsA = colp.tile([P, tpb], F32, tag="rsA")
                nc.vector.reciprocal(rsA[:], nrmA[:])
                argA = colp.tile([P, tpb], F32, tag="argA")
                nc.vector.tensor_tensor(
                    out=argA[:], in0=dotA[:], in1=rsA[:], op=mybir.AluOpType.mult
                )
                wA = colp.tile([P, tpb, 1], BF16, tag="wA")
                nc.scalar.activation(
                    out=wA[:, :, 0], in_=argA[:],
                    func=mybir.ActivationFunctionType.Exp, scale=beta_sb[:],
                )
                # ---- block-wide one-hot * w
                pwA = pwp.tile([P, tpb, P], BF16, tag="pwA")
                nc.vector.tensor_tensor(
                    out=prod[:], in0=iota_bf[:][:, None, :].to_broadcast([P, tpb, P]),
                    in1=dblk[:].to_broadcast([P, tpb, P]),
                    op=mybir.AluOpType.is_equal,
                )
                nc.vector.tensor_tensor(
                    out=pwA[:], in0=prod[:],
                    in1=wA[:].to_broadcast([P, tpb, P]),
                    op=mybir.AluOpType.mult,
                )

                psum = psump.tile([P, D + 1], F32, tag="acc")

                for t in range(tpb):
                    if t < tpb_lo:
                        xs = xs_lo[:, t, :]
                    else:
                        xs = xs_hi[:, t - tpb_lo, :]
                    nc.tensor.matmul(
                        out=psum[:, 0:D], lhsT=pwA[:, t, :], rhs=xs,
                        start=(t == 0), stop=False, skip_group_check=True,
                    )
                    nc.tensor.matmul(
                        out=psum[:, D : D + 1], lhsT=pwA[:, t, :], rhs=ones_bf[:],
                        start=False, stop=(t == tpb - 1), skip_group_check=True,
                    )

                # epilogue: rows = relu(M / s); scatter rows to out
                s_safe = colp.tile([P, 1], F32, tag="ssafe")
                nc.vector.tensor_scalar(
                    out=s_safe[:], in0=psum[:, D : D + 1], scalar1=1e-30,
                    scalar2=None, op0=mybir.AluOpType.max,
                )
                sinv = colp.tile([P, 1], F32, tag="sinv")
                nc.vector.reciprocal(sinv[:], s_safe[:])
                orow = orowp.tile([P, D], F32, tag="orow")
                nc.vector.tensor_scalar(
                    out=orow[:], in0=psum[:, 0:D], scalar1=sinv[:], scalar2=0.0,
                    op0=mybir.AluOpType.mult, op1=mybir.AluOpType.max,
                )
                # scalar's DMA queue: keeps the epilogue-gated write out of
                # the sync FIFO so it can't head-of-line-block later idx loads
                nc.scalar.dma_start(
                    out=out_ext[b * P : (b + 1) * P, :], in_=orow[:]
                )

    nc.compile()
    return nc


def _wrap16(vals, ncols, pad):
    """[n] -> [128, ncols] int16 in dma_gather's 16-wrapped, 8x-replicated
    partition layout (idx j at [j%16, j//16])."""
    full = np.full(ncols * 16, pad, np.int64)
    full[: len(vals)] = vals
    w = full.reshape(ncols, 16).T.astype(np.int16)  # [16, ncols]
    return np.tile(w, (8, 1))


def _decompose(dst_sorted, src_sorted, N, tpb_lo, tpb_hi, hi_base, max_nodes=P):
    """Blocks of consecutive nodes with <=max_nodes nodes, <=tpb_lo*128
    low-src edges and <=tpb_hi*128 high-src edges."""
    deg = np.bincount(dst_sorted, minlength=N)
    deg_lo = np.bincount(dst_sorted[src_sorted < hi_base], minlength=N)
    deg_hi = deg - deg_lo
    cap_lo, cap_hi = tpb_lo * P, tpb_hi * P
    assert deg_lo.max() <= cap_lo and deg_hi.max() <= cap_hi
    blocks = []
    n0 = e0 = 0
    lo = hi = 0
    for node in range(N):
        dl, dh = int(deg_lo[node]), int(deg_hi[node])
        if (node - n0) >= max_nodes or lo + dl > cap_lo or hi + dh > cap_hi:
            blocks.append((n0, node, e0, e0 + lo + hi))
            n0, e0 = node, e0 + lo + hi
            lo = hi = 0
        lo += dl
        hi += dh
    blocks.append((n0, N, e0, e0 + lo + hi))
    return blocks


def _prep_inputs(x, beta, edge_index, N, D, tpb_lo, tpb_hi, hi_base, ncores):
    tpb = tpb_lo + tpb_hi
    gl, gh, gt = tpb_lo * 8, tpb_hi * 8, tpb * 8
    loop = np.arange(N, dtype=np.int64)
    src = np.concatenate([np.asarray(edge_index[0]), loop]).astype(np.int64)
    dst = np.concatenate([np.asarray(edge_index[1]), loop]).astype(np.int64)
    order = np.argsort(dst, kind="stable")
    src_s = src[order]
    dst_s = dst[order]

    blocks = _decompose(dst_s, src_s, N, tpb_lo, tpb_hi, hi_base)
    nbt = len(blocks)
    sizes = [nbt // ncores + (1 if i < nbt % ncores else 0) for i in range(ncores)]
    NB = max(sizes)

    core_blocks, bpos = [], 0
    for k in range(ncores):
        core_blocks.append(blocks[bpos : bpos + sizes[k]])
        bpos += sizes[k]
    ranges = [
        (blks[0][0], blks[-1][1]) if blks else (0, 0) for blks in core_blocks
    ]
    ndst = max(hi - lo for lo, hi in ranges)
    ndst = (ndst + 1023) // 1024 * 1024  # prologue supertile multiple
    assert ndst <= 32767

    xf32 = np.ascontiguousarray(np.asarray(x), dtype=np.float32)
    beta128 = np.full((P, 1), float(np.asarray(beta).reshape(-1)[0]), np.float32)

    in_maps = []
    for k in range(ncores):
        blks = core_blocks[k]
        lo_k = ranges[k][0]
        a_sl = np.zeros((P, NB * gl), np.int16)
        a_sh = np.zeros((P, NB * gh), np.int16)
        a_di = np.zeros((P, NB * gt), np.int16)
        a_blk = np.full((tpb * NB, P), BLK_SENTINEL, np.float32)
        for bi, (n0, n1, e0, e1) in enumerate(blks):
            s = src_s[e0:e1]
            d = dst_s[e0:e1]
            lomask = s < hi_base
            s_lo, d_lo = s[lomask], d[lomask]
            s_hi, d_hi = s[~lomask] - hi_base, d[~lomask]
            nlo, nhi = len(s_lo), len(s_hi)
            a_sl[:, bi * gl : (bi + 1) * gl] = _wrap16(s_lo, gl, 0)
            a_sh[:, bi * gh : (bi + 1) * gh] = _wrap16(s_hi, gh, 0)
            # combined order: lo edges in slots [0, tpb_lo*128), hi after
            dcomb = np.zeros(tpb * P, np.int64)
            dcomb[:nlo] = d_lo - lo_k
            dcomb[tpb_lo * P : tpb_lo * P + nhi] = d_hi - lo_k
            a_di[:, bi * gt : (bi + 1) * gt] = _wrap16(dcomb, gt, 0)
            bcomb = np.full(tpb * P, BLK_SENTINEL, np.float32)
            bcomb[:nlo] = (d_lo - n0).astype(np.float32)
            bcomb[tpb_lo * P : tpb_lo * P + nhi] = (d_hi - n0).astype(np.float32)
            a_blk[bi * tpb : (bi + 1) * tpb] = bcomb.reshape(tpb, P)
        xdst = np.zeros((ndst, D), np.float32)
        span = ranges[k][1] - lo_k
        xdst[:span] = xf32[lo_k : ranges[k][1]]
        a_idx = np.zeros((P, NB * (gl + gh + gt)), np.int16)
        ga = gl + gh + gt
        for bi in range(NB):
            a_idx[:, bi * ga : bi * ga + gl] = a_sl[:, bi * gl : (bi + 1) * gl]
            a_idx[:, bi * ga + gl : bi * ga + gl + gh] = a_sh[:, bi * gh : (bi + 1) * gh]
            a_idx[:, bi * ga + gl + gh : (bi + 1) * ga] = a_di[:, bi * gt : (bi + 1) * gt]
        in_maps.append(
            {
                "x": xf32,
                "xdst": xdst,
                "idx_all": a_idx,
                "dst_blk": np.ascontiguousarray(a_blk.T).astype(ml_dtypes.bfloat16),
                "beta128": beta128,
            }
        )
    return in_maps, ranges, NB, ndst, core_blocks


def _enable_axon_ntff():
    """Install the NTFF profile hook that the stub antenv package lacks."""
    import sys, types
    try:
        import antenv

        if "antenv.axon_hooks" not in sys.modules:
            mod = types.ModuleType("antenv.axon_hooks")
            mod._hook = None
            mod.set_axon_ntff_profile_hook = lambda h: setattr(mod, "_hook", h)
            mod.get_axon_ntff_profile_hook = lambda: mod._hook
            sys.modules["antenv.axon_hooks"] = mod
            antenv.axon_hooks = mod
            from trn_agent_boot.trn_boot import _ntff_profile_via_ctypes

            mod._hook = _ntff_profile_via_ctypes("/opt/axon/libaxon_pjrt.so")
        import concourse.bass_utils as bu

        bu.upload_artifacts = lambda tmpdir: tmpdir
        return True
    except Exception as e:
        print(f"ntff hook install failed: {e}")
        return False


def _run(x, beta, edge_index, trace=False):
    from concourse.bass_utils import run_bass_kernel_spmd

    if trace:
        trace = _enable_axon_ntff()
    N, D = x.shape
    in_maps, ranges, NB, ndst, core_blocks = _prep_inputs(
        x, beta, edge_index, N, D, TPB_LO, TPB_HI, HI_BASE, NCORES
    )
    nc = _build_graph(N, D, NB, TPB_LO, TPB_HI, ndst, HI_BASE)
    res = run_bass_kernel_spmd(
        nc, in_maps, core_ids=list(range(NCORES)), trace=trace
    )
    out = np.zeros((N, D), np.float32)
    for k in range(NCORES):
        co = res.results[k]["out"]
        for bi, (n0, n1, e0, e1) in enumerate(core_blocks[k]):
            out[n0:n1] = co[bi * P : bi * P + (n1 - n0)]
    return out, res


def kernel(x, beta, edge_index):
    out, _ = _run(
        np.asarray(x), np.asarray(beta), np.asarray(edge_index), trace=False
    )
    return out


# revision 31
# speedup vs baseline: 1.2155x; 1.0258x over previous
"""AGNN layer (cosine-attention message passing) on 8 TRN2 NeuronCores.

Host sharding: append self-loops, sort edges by destination node, cut the node
range into blocks (<=128 nodes, bounded edge count), hand contiguous block
runs to the 8 cores. Every softmax segment then lives on one core: no
collectives anywhere.

Device kernel per core (identical SPMD graph, per-core data):
  prologue: cast x (f32) -> bf16 src gather table; normalize the core's own
            node-range slice (xdst input) -> unit-norm bf16 dst table.
  per block: dma_gather (4 SWDGE queues round-robin, <=1024 rows/call)
            fetches per-edge rows: src rows from the low half [0,32768) and
            high half [32768,N) of the table (int16 index limit forces the
            static lo/hi tile split), dst rows from the per-core dst table
            (relative indices). Block-wide DVE/ACT ops compute per-edge
            cos = (x_s . xn_d) * rsqrt(|x_s|^2) and w = exp(beta*cos), and
            build the one-hot(dst)*w matrix via iota/is_equal; per 128-edge
            tile, two PSUM-accumulating matmuls build per-node sum(w*x_src)
            and sum(w).
  per block epilogue: rows = relu(M/s) written to a compact per-block
            output; the host scatters block rows back to node order.

Logits are cosines scaled by beta (bounded), so exp never overflows and the
reference's segment-max subtraction cancels exactly -- single pass suffices.
"""

import numpy as np
import ml_dtypes

import concourse.bacc as bacc
import concourse.mybir as mybir
import concourse.tile as tile
from concourse.library_config import mlp as _mlp_lib
import concourse.tile_sem_assignment as _tsa

# Tile's DMASW-lane round-robin is SWDGE-queue-oblivious; a completion sem
# shared by two queues desyncs the ucode's per-queue ring-reclaim waits.
# Partition the 8 lanes so queue q owns lanes {2q, 2q+1}.
_orig_assign_tick = _tsa.TileClockTick._assign_tick


def _assign_tick_queue_aware(self, inst):
    if (
        isinstance(inst, mybir.InstDMAGatherAnt)
        and inst.engine == mybir.EngineType.Pool
    ):
        qn = inst.queue_num
        if not hasattr(self, "_qcnt"):
            self._qcnt = {}
        cnt = self._qcnt.get(qn, 0)
        self._qcnt[qn] = cnt + 1
        self.next_sw_dma_idx = qn * 2 + (cnt % 2)
    return _orig_assign_tick(self, inst)


_tsa.TileClockTick._assign_tick = _assign_tick_queue_aware

P = 128
N_NODES = 50000
D_FEAT = 128
NCORES = 8
HI_BASE = 32768  # int16 gather index limit
TPB_LO = 23  # tiles per block for src<HI_BASE edges
TPB_HI = 12  # tiles per block for src>=HI_BASE edges
BLK_SENTINEL = 300.0

F32 = mybir.dt.float32
BF16 = mybir.dt.bfloat16
I16 = mybir.dt.int16


def _build_graph(N, D, NB, tpb_lo, tpb_hi, ndst, hi_base):
    tpb = tpb_lo + tpb_hi
    gl, gh, gt = tpb_lo * 8, tpb_hi * 8, tpb * 8  # idx cols (16-wrapped)
    nc = bacc.Bacc(
        "TRN2", target_bir_lowering=False, debug=False, enable_asserts=False,
        num_swdge_queues=4,
    )
    x_ext = nc.dram_tensor("x", [N, D], F32, kind="ExternalInput").ap()
    xdst_ext = nc.dram_tensor("xdst", [ndst, D], F32, kind="ExternalInput").ap()
    ga = gl + gh + gt
    idx_all = nc.dram_tensor("idx_all", [P, NB * ga], I16, kind="ExternalInput").ap()
    dst_blk = nc.dram_tensor("dst_blk", [P, NB * tpb], BF16, kind="ExternalInput").ap()
    beta128 = nc.dram_tensor("beta128", [P, 1], F32, kind="ExternalInput").ap()
    out_ext = nc.dram_tensor("out", [NB * P, D], F32, kind="ExternalOutput").ap()

    n_lo = min(hi_base, N)
    n_hi = max(N - hi_base, 8)
    xb_lo = nc.dram_tensor("xb_lo", [n_lo, D], BF16).ap()
    xb_hi = nc.dram_tensor("xb_hi", [n_hi, D], BF16).ap()
    xd_tab = nc.dram_tensor("xd_table", [ndst, D], BF16).ap()

    with tile.TileContext(nc) as tc:
        with (
            tc.tile_pool(name="const", bufs=1) as constp,
            tc.tile_pool(name="prolog", bufs=3) as prologp,
            tc.tile_pool(name="idx", bufs=3) as idxp,
            tc.tile_pool(name="gsl", bufs=3) as gslp,
            tc.tile_pool(name="gsh", bufs=3) as gshp,
            tc.tile_pool(name="gd", bufs=3) as gdp,
            tc.tile_pool(name="work", bufs=2) as workp,
            tc.tile_pool(name="pw", bufs=3) as pwp,
            tc.tile_pool(name="cols", bufs=4) as colp,
            tc.tile_pool(name="orow", bufs=2) as orowp,
            tc.tile_pool(name="psum", bufs=4, space="PSUM") as psump,
        ):
            nc.gpsimd.load_library(_mlp_lib)

            # ---- constants ----
            iota_i16 = constp.tile([P, P], I16)
            nc.gpsimd.iota(iota_i16[:], pattern=[[1, P]], base=0, channel_multiplier=0)
            iota_bf = constp.tile([P, P], BF16)
            nc.vector.tensor_copy(iota_bf[:], iota_i16[:])
            ones_bf = constp.tile([P, 1], BF16)
            nc.vector.memset(ones_bf[:], 1.0)
            beta_sb = constp.tile([P, 1], F32)
            nc.sync.dma_start(out=beta_sb[:], in_=beta128[:, :])

            # ---- prologue: cast x -> bf16 tables ----
            def cast_table(src_ap, dst_ap, nrows):
                r0 = 0
                while r0 < nrows:
                    rows = min(2048, nrows - r0)
                    rpp = 16
                    while rows % rpp:
                        rpp //= 2
                    pp = rows // rpp
                    xt = prologp.tile([P, 16, D], F32, tag="xt")
                    nc.sync.dma_start(
                        out=xt[:pp, 0:rpp, :], in_=src_ap[r0 : r0 + rows, :]
                    )
                    xbt = prologp.tile([P, 16, D], BF16, tag="xbt")
                    nc.vector.tensor_copy(xbt[:pp, 0:rpp, :], xt[:pp, 0:rpp, :])
                    nc.scalar.dma_start(
                        out=dst_ap[r0 : r0 + rows, :], in_=xbt[:pp, 0:rpp, :]
                    )
                    r0 += rows

            cast_table(x_ext, xb_lo, n_lo)
            if N > hi_base:
                cast_table(x_ext[hi_base:N, :], xb_hi, N - hi_base)

            # dst table is pre-normalized: xd_tab rows = x / |x|
            r0 = 0
            while r0 < ndst:
                rows = min(1024, ndst - r0)
                pp = rows // 8
                xt = prologp.tile([P, 8, D], F32, tag="xt")
                nc.sync.dma_start(out=xt[:pp], in_=xdst_ext[r0 : r0 + rows, :])
                sqv = prologp.tile([P, 8, D], F32, tag="sqv")
                nc.vector.tensor_tensor(
                    out=sqv[:pp], in0=xt[:pp], in1=xt[:pp], op=mybir.AluOpType.mult
                )
                ssum = prologp.tile([P, 8], F32, tag="ssum")
                nc.vector.tensor_reduce(
                    out=ssum[:pp], in_=sqv[:pp], axis=mybir.AxisListType.X,
                    op=mybir.AluOpType.add,
                )
                # rows of zeros (tail padding) -> clamp to avoid inf
                nsafe = prologp.tile([P, 8], F32, tag="nsafe")
                nc.vector.tensor_scalar(
                    out=nsafe[:pp], in0=ssum[:pp], scalar1=1e-30, scalar2=None,
                    op0=mybir.AluOpType.max,
                )
                nrmv = prologp.tile([P, 8], F32, tag="nrmv")
                nc.scalar.activation(
                    out=nrmv[:pp], in_=nsafe[:pp],
                    func=mybir.ActivationFunctionType.Sqrt,
                )
                rcp = prologp.tile([P, 8, 1], F32, tag="rcp")
                nc.vector.reciprocal(rcp[:pp, :, 0], nrmv[:pp])
                xnt = prologp.tile([P, 8, D], BF16, tag="xnt")
                nc.vector.tensor_tensor(
                    out=xnt[:pp], in0=xt[:pp],
                    in1=rcp[:pp].to_broadcast([pp, 8, D]),
                    op=mybir.AluOpType.mult,
                )
                nc.scalar.dma_start(out=xd_tab[r0 : r0 + rows, :], in_=xnt[:pp])
                r0 += rows

            # ---- edge loop ----
            self_q = [0]
            for b in range(NB):
                idxt = idxp.tile([P, ga], I16, tag="idxt")
                nc.sync.dma_start(out=idxt[:], in_=idx_all[:, b * ga : (b + 1) * ga])
                sl = idxt[:, 0:gl]
                sh = idxt[:, gl : gl + gh]
                di = idxt[:, gl + gh : ga]
                dblk = idxp.tile([P, tpb, 1], BF16, tag="dblk")
                nc.sync.dma_start(
                    out=dblk[:], in_=dst_blk[:, b * tpb : (b + 1) * tpb]
                )

                def gather_rows(out_tile, tab_ap, idx_tile, total):
                    # SWDGE descriptor-ring capacity caps one call at ~1024 rows;
                    # round-robin the 4 queues so all Q7 pairs generate descs
                    off = 0
                    while off < total:
                        ni = min(1024, total - off)
                        nc.gpsimd.dma_gather(
                            out_tile[:, off // P : (off + ni) // P, :],
                            tab_ap,
                            idx_tile[:, off // 16 : (off + ni) // 16],
                            ni, ni, D,
                            queue_num=self_q[0] % 4,
                        )
                        self_q[0] += 1
                        off += ni

                xs_lo = gslp.tile([P, tpb_lo, D], BF16, tag="xsl")
                gather_rows(xs_lo, xb_lo[:, :], sl, tpb_lo * P)
                xs_hi = gshp.tile([P, tpb_hi, D], BF16, tag="xsh")
                gather_rows(xs_hi, xb_hi[:, :], sh, tpb_hi * P)
                xd_blk = gdp.tile([P, tpb, D], BF16, tag="xd")
                gather_rows(xd_blk, xd_tab[:, :], di, tpb * P)

                # ---- block-wide logits: cos = dot * rsqrt(ss); |xd| == 1
                prod = workp.tile([P, tpb, D], BF16, tag="prod")
                ssA = colp.tile([P, tpb], BF16, tag="ssA")
                nc.scalar.activation(
                    out=prod[:, 0:tpb_lo, :], in_=xs_lo[:],
                    func=mybir.ActivationFunctionType.Square,
                )
                nc.scalar.activation(
                    out=prod[:, tpb_lo:tpb, :], in_=xs_hi[:],
                    func=mybir.ActivationFunctionType.Square,
                )
                with nc.allow_low_precision(reason="bf16 stats, tol 2e-2"):
                    nc.vector.tensor_reduce(
                        out=ssA[:], in_=prod[:], axis=mybir.AxisListType.X,
                        op=mybir.AluOpType.add,
                    )
                dotA = colp.tile([P, tpb], BF16, tag="dotA")
                nc.vector.tensor_tensor(
                    out=prod[:, 0:tpb_lo, :], in0=xs_lo[:],
                    in1=xd_blk[:, 0:tpb_lo, :], op=mybir.AluOpType.mult,
                )
                nc.vector.tensor_tensor(
                    out=prod[:, tpb_lo:tpb, :], in0=xs_hi[:],
                    in1=xd_blk[:, tpb_lo:tpb, :], op=mybir.AluOpType.mult,
                )
                with nc.allow_low_precision(reason="bf16 stats, tol 2e-2"):
                    nc.vector.tensor_reduce(
                        out=dotA[:], in_=prod[:], axis=mybir.AxisListType.X,
                        op=mybir.AluOpType.add,
                    )
                nrmA = colp.tile([P, tpb], F32, tag="nrmA")
                nc.scalar.activation(
                    out=nrmA[:], in_=ssA[:],
                    func=mybir.ActivationFunctionType.Sqrt,
                )
                rsA = colp.tile([P, tpb], F32, tag="rsA")
                nc.vector.reciprocal(rsA[:], nrmA[:])
                argA = colp.tile([P, tpb], F32, tag="argA")
                nc.vector.tensor_tensor(
                    out=argA[:], in0=dotA[:], in1=rsA[:], op=mybir.AluOpType.mult
                )
                wA = colp.tile([P, tpb, 1], BF16, tag="wA")
                nc.scalar.activation(
                    out=wA[:, :, 0], in_=argA[:],
                    func=mybir.ActivationFunctionType.Exp, scale=beta_sb[:],
                )
                # ---- block-wide one-hot * w
                pwA = pwp.tile([P, tpb, P], BF16, tag="pwA")
                nc.vector.tensor_tensor(
                    out=prod[:], in0=iota_bf[:][:, None, :].to_broadcast([P, tpb, P]),
                    in1=dblk[:].to_broadcast([P, tpb, P]),
                    op=mybir.AluOpType.is_equal,
                )
                nc.vector.tensor_tensor(
                    out=pwA[:], in0=prod[:],
                    in1=wA[:].to_broadcast([P, tpb, P]),
                    op=mybir.AluOpType.mult,
                )

                psum = psump.tile([P, D + 1], F32, tag="acc")

                for t in range(tpb):
                    if t < tpb_lo:
                        xs = xs_lo[:, t, :]
                    else:
                        xs = xs_hi[:, t - tpb_lo, :]
                    nc.tensor.matmul(
                        out=psum[:, 0:D], lhsT=pwA[:, t, :], rhs=xs,
                        start=(t == 0), stop=False, skip_group_check=True,
                    )
                    nc.tensor.matmul(
                        out=psum[:, D : D + 1], lhsT=pwA[:, t, :], rhs=ones_bf[:],
                        start=False, stop=(t == tpb - 1), skip_group_check=True,
                    )

                # epilogue: rows = relu(M / s); scatter rows to out
                s_safe = colp.tile([P, 1], F32, tag="ssafe")
                nc.vector.tensor_scalar(
                    out=s_safe[:], in0=psum[:, D : D + 1], scalar1=1e-30,
                    scalar2=None, op0=mybir.AluOpType.max,
                )
                sinv = colp.tile([P, 1], F32, tag="sinv")
                nc.vector.reciprocal(sinv[:], s_safe[:])
                orow = orowp.tile([P, D], F32, tag="orow")
                nc.vector.tensor_scalar(
                    out=orow[:], in0=psum[:, 0:D], scalar1=sinv[:], scalar2=0.0,
                    op0=mybir.AluOpType.mult, op1=mybir.AluOpType.max,
                )
                # scalar's DMA queue: keeps the epilogue-gated write out of
                # the sync FIFO so it can't head-of-line-block later idx loads
                nc.scalar.dma_start(
                    out=out_ext[b * P : (b + 1) * P, :], in_=orow[:]
                )

    nc.compile()
    return nc


def _wrap16(vals, ncols, pad):
    """[n] -> [128, ncols] int16 in dma_gather's 16-wrapped, 8x-replicated
    partition layout (idx j at [j%16, j//16])."""
    full = np.full(ncols * 16, pad, np.int64)
    full[: len(vals)] = vals
    w = full.reshape(ncols, 16).T.astype(np.int16)  # [16, ncols]
    return np.tile(w, (8, 1))


def _decompose(dst_sorted, src_sorted, N, tpb_lo, tpb_hi, hi_base, max_nodes=P):
    """Blocks of consecutive nodes with <=max_nodes nodes, <=tpb_lo*128
    low-src edges and <=tpb_hi*128 high-src edges."""
    deg = np.bincount(dst_sorted, minlength=N)
    deg_lo = np.bincount(dst_sorted[src_sorted < hi_base], minlength=N)
    deg_hi = deg - deg_lo
    cap_lo, cap_hi = tpb_lo * P, tpb_hi * P
    assert deg_lo.max() <= cap_lo and deg_hi.max() <= cap_hi
    blocks = []
    n0 = e0 = 0
    lo = hi = 0
    for node in range(N):
        dl, dh = int(deg_lo[node]), int(deg_hi[node])
        if (node - n0) >= max_nodes or lo + dl > cap_lo or hi + dh > cap_hi:
            blocks.append((n0, node, e0, e0 + lo + hi))
            n0, e0 = node, e0 + lo + hi
            lo = hi = 0
        lo += dl
        hi += dh
    blocks.append((n0, N, e0, e0 + lo + hi))
    return blocks


def _prep_inputs(x, beta, edge_index, N, D, tpb_lo, tpb_hi, hi_base, ncores):
    tpb = tpb_lo + tpb_hi
    gl, gh, gt = tpb_lo * 8, tpb_hi * 8, tpb * 8
    loop = np.arange(N, dtype=np.int64)
    src = np.concatenate([np.asarray(edge_index[0]), loop]).astype(np.int64)
    dst = np.concatenate([np.asarray(edge_index[1]), loop]).astype(np.int64)
    order = np.argsort(dst, kind="stable")
    src_s = src[order]
    dst_s = dst[order]

    blocks = _decompose(dst_s, src_s, N, tpb_lo, tpb_hi, hi_base)
    nbt = len(blocks)
    sizes = [nbt // ncores + (1 if i < nbt % ncores else 0) for i in range(ncores)]
    NB = max(sizes)

    core_blocks, bpos = [], 0
    for k in range(ncores):
        core_blocks.append(blocks[bpos : bpos + sizes[k]])
        bpos += sizes[k]
    ranges = [
        (blks[0][0], blks[-1][1]) if blks else (0, 0) for blks in core_blocks
    ]
    ndst = max(hi - lo for lo, hi in ranges)
    ndst = (ndst + 1023) // 1024 * 1024  # prologue supertile multiple
    assert ndst <= 32767

    xf32 = np.ascontiguousarray(np.asarray(x), dtype=np.float32)
    beta128 = np.full((P, 1), float(np.asarray(beta).reshape(-1)[0]), np.float32)

    in_maps = []
    for k in range(ncores):
        blks = core_blocks[k]
        lo_k = ranges[k][0]
        a_sl = np.zeros((P, NB * gl), np.int16)
        a_sh = np.zeros((P, NB * gh), np.int16)
        a_di = np.zeros((P, NB * gt), np.int16)
        a_blk = np.full((tpb * NB, P), BLK_SENTINEL, np.float32)
        for bi, (n0, n1, e0, e1) in enumerate(blks):
            s = src_s[e0:e1]
            d = dst_s[e0:e1]
            lomask = s < hi_base
            s_lo, d_lo = s[lomask], d[lomask]
            s_hi, d_hi = s[~lomask] - hi_base, d[~lomask]
            nlo, nhi = len(s_lo), len(s_hi)
            a_sl[:, bi * gl : (bi + 1) * gl] = _wrap16(s_lo, gl, 0)
            a_sh[:, bi * gh : (bi + 1) * gh] = _wrap16(s_hi, gh, 0)
            # combined order: lo edges in slots [0, tpb_lo*128), hi after
            dcomb = np.zeros(tpb * P, np.int64)
            dcomb[:nlo] = d_lo - lo_k
            dcomb[tpb_lo * P : tpb_lo * P + nhi] = d_hi - lo_k
            a_di[:, bi * gt : (bi + 1) * gt] = _wrap16(dcomb, gt, 0)
            bcomb = np.full(tpb * P, BLK_SENTINEL, np.float32)
            bcomb[:nlo] = (d_lo - n0).astype(np.float32)
            bcomb[tpb_lo * P : tpb_lo * P + nhi] = (d_hi - n0).astype(np.float32)
            a_blk[bi * tpb : (bi + 1) * tpb] = bcomb.reshape(tpb, P)
        xdst = np.zeros((ndst, D), np.float32)
        span = ranges[k][1] - lo_k
        xdst[:span] = xf32[lo_k : ranges[k][1]]
        a_idx = np.zeros((P, NB * (gl + gh + gt)), np.int16)
        ga = gl + gh + gt
        for bi in range(NB):
            a_idx[:, bi * ga : bi * ga + gl] = a_sl[:, bi * gl : (bi + 1) * gl]
            a_idx[:, bi * ga + gl : bi * ga + gl + gh] = a_sh[:, bi * gh : (bi + 1) * gh]
            a_idx[:, bi * ga + gl + gh : (bi + 1) * ga] = a_di[:, bi * gt : (bi + 1) * gt]
        in_maps.append(
            {
                "x": xf32,
                "xdst": xdst,
                "idx_all": a_idx,
                "dst_blk": np.ascontiguousarray(a_blk.T).astype(ml_dtypes.bfloat16),
                "beta128": beta128,
            }
        )
    return in_maps, ranges, NB, ndst, core_blocks


def _enable_axon_ntff():
    """Install the NTFF profile hook that the stub antenv package lacks."""
    import sys, types
    try:
        import antenv

        if "antenv.axon_hooks" not in sys.modules:
            mod = types.ModuleType("antenv.axon_hooks")
            mod._hook = None
            mod.set_axon_ntff_profile_hook = lambda h: setattr(mod, "_hook", h)
            mod.get_axon_ntff_profile_hook = lambda: mod._hook
            sys.modules["antenv.axon_hooks"] = mod
            antenv.axon_hooks = mod
            from trn_agent_boot.trn_boot import _ntff_profile_via_ctypes

            mod._hook = _ntff_profile_via_ctypes("/opt/axon/libaxon_pjrt.so")
        import concourse.bass_utils as bu

        bu.upload_artifacts = lambda tmpdir: tmpdir
        return True
    except Exception as e:
        print(f"ntff hook install failed: {e}")
        return False


def _run(x, beta, edge_index, trace=False):
    from concourse.bass_utils import run_bass_kernel_spmd

    if trace:
        trace = _enable_axon_ntff()
    N, D = x.shape
    in_maps, ranges, NB, ndst, core_blocks = _prep_inputs(
        x, beta, edge_index, N, D, TPB_LO, TPB_HI, HI_BASE, NCORES
    )
    nc = _build_graph(N, D, NB, TPB_LO, TPB_HI, ndst, HI_BASE)
    res = run_bass_kernel_spmd(
        nc, in_maps, core_ids=list(range(NCORES)), trace=trace
    )
    out = np.zeros((N, D), np.float32)
    for k in range(NCORES):
        co = res.results[k]["out"]
        for bi, (n0, n1, e0, e1) in enumerate(core_blocks[k]):
            out[n0:n1] = co[bi * P : bi * P + (n1 - n0)]
    return out, res


def kernel(x, beta, edge_index):
    out, _ = _run(
        np.asarray(x), np.asarray(beta), np.asarray(edge_index), trace=False
    )
    return out
